# revision 10
# baseline (speedup 1.0000x reference)
"""DeepAR LSTM kernel for 8 Trainium2 NeuronCores.

Data-parallel over batch (256 -> 8 cores x 32). Latency-oriented design:
the recurrence is a serial chain PE -> ACT -> DVE -> ACT -> DVE per step,
so everything else is moved off that chain.

  * fp16 matmul operands everywhere (1 PE cycle/row vs 4 for fp32);
    PSUM accumulation stays fp32.
  * Gate pre-activations accumulate in a persistent 16-step PSUM ring
    [128, 16*4*32]. The x-side matmuls for step t depend only on x (DMA'd
    up front), so the PE executes them early; only the 4 h-side matmuls
    (start=False accumulate) sit on the critical path.
  * One sigmoid covers all 4 gates: the g-gate rows of W/bias are
    pre-scaled by 2 on the host, and tanh(a) = 2*sigmoid(2a) - 1 is fixed
    up in DVE with a single tensor_scalar op.
  * Bias is folded into the x-side matmul via a constant-1 row of x.
  * mu/logsigma heads run as one PE matmul per step (stationary = h_t),
    accumulating into a PSUM ring that is DMA'd straight to DRAM every
    4 steps; head biases are added on the host.
"""

import os
import sys
from contextlib import ExitStack

import numpy as np

sys.path.insert(0, "/opt/trn_rl_repo")

import concourse.bass as bass
import concourse.tile as tile
from concourse import bacc, mybir
from concourse.bass_utils import run_bass_kernel_spmd

L, B, IN, K, OBS = 1024, 256, 64, 128, 32
NCORES = 8
BL = B // NCORES   # 32 batch rows per core
SLOTS = 16         # gate-psum ring depth (steps)
HSLOTS = 8         # heads-psum ring depth (steps)

_LSTEPS = int(os.environ.get("KERNEL_LSTEPS", L))  # smoke-test override

F32 = mybir.dt.float32
F16 = mybir.dt.float16
AF = mybir.ActivationFunctionType
OP = mybir.AluOpType

_cache = {}
RUN_KW = {}         # test harness may inject trace=True/tmpdir
LAST_RESULT = None  # BassKernelResults of the most recent run


def build_nc(nsteps: int) -> bass.Bass:
    nc = bacc.Bacc(
        "TRN2", target_bir_lowering=False, debug=False, num_devices=NCORES
    )
    xt = nc.dram_tensor("xt", [IN + 1, nsteps * BL], F16, kind="ExternalInput")
    whh = nc.dram_tensor("whh_t", [K, 4 * K], F16, kind="ExternalInput")
    wih = nc.dram_tensor("wih_t", [IN + 1, 4 * K], F16, kind="ExternalInput")
    whd = nc.dram_tensor("wheads", [K, 2 * OBS], F16, kind="ExternalInput")
    heads = nc.dram_tensor(
        "heads", [BL, nsteps * 2 * OBS], F32, kind="ExternalOutput"
    )

    with ExitStack() as ctx:
        tc = ctx.enter_context(tile.TileContext(nc))
        singles = ctx.enter_context(tc.tile_pool(name="singles", bufs=1))
        gpsp = ctx.enter_context(tc.tile_pool(name="gps", bufs=1, space="PSUM"))
        hpsp = ctx.enter_context(tc.tile_pool(name="hps", bufs=1, space="PSUM"))
        dpsp = ctx.enter_context(tc.tile_pool(name="dps", bufs=1, space="PSUM"))

        whh_sb = singles.tile([K, 4 * K], F16)
        nc.sync.dma_start(whh_sb[:], whh[:])
        wih_sb = singles.tile([IN + 1, 4 * K], F16)
        nc.sync.dma_start(wih_sb[:], wih[:])
        whd_sb = singles.tile([K, 2 * OBS], F16)
        nc.sync.dma_start(whd_sb[:], whd[:])
        xt_sb = singles.tile([IN + 1, nsteps * BL], F16)
        nc.sync.dma_start(xt_sb[:], xt[:])

        sgt = [singles.tile([K, 4 * BL], F32, name=f"sg{i}") for i in range(2)]
        ct = [singles.tile([K, BL], F32, name=f"c{i}") for i in range(2)]
        tht = [singles.tile([K, BL], F32, name=f"th{i}") for i in range(2)]
        ht = [singles.tile([K, BL], F16, name=f"h{i}") for i in range(2)]
        g2 = singles.tile([K, BL], F32)
        ig = singles.tile([K, BL], F32)
        fc = singles.tile([K, BL], F32)
        stgt = [
            singles.tile([BL, 4 * 2 * OBS], F32, name=f"stg{i}")
            for i in range(2)
        ]

        gates_ps = gpsp.tile([K, SLOTS * 4 * BL], F32)    # 4 PSUM banks
        heads_ps = hpsp.tile([BL, HSLOTS * 2 * OBS], F32)  # 1 PSUM bank

        # A matmul can carry only ONE sync wait; make PE observe each DMA
        # semaphore via a throwaway 1x1 matmul so real matmuls never need
        # a DMA wait on top of a data-dependency wait.
        dummy_ps = dpsp.tile([1, 1], F32)
        absorb_state = {"first": True}

        def pe_absorb(tile_ap):
            nc.tensor.matmul(
                dummy_ps[:], tile_ap[0:1, 0:1], tile_ap[0:1, 0:1],
                start=absorb_state["first"], stop=False,
                skip_group_check=True,
            )
            absorb_state["first"] = False

        pe_absorb(whh_sb)
        pe_absorb(wih_sb)
        pe_absorb(whd_sb)
        pe_absorb(xt_sb)

        last_flush = -1
        for t in range(nsteps):
            base = (t % SLOTS) * 4 * BL
            xs = xt_sb[:, t * BL : (t + 1) * BL]
            # x-side matmuls: no h dependency -> execute early, off the
            # critical path. First one carries the WAR wait vs the sigmoid
            # read of this slot 16 steps ago.
            #
            # start=True marks the whole 2KB PSUM bank (4 slots) as
            # pending-zero; writes to marked bytes overwrite and clear the
            # mark, writes to cleared bytes accumulate. So assert start
            # only on the first matmul touching each bank (t%4==0, g==0);
            # every other x-matmul overwrites via its still-pending mark
            # and every h-matmul accumulates.
            for g in range(4):
                nc.tensor.matmul(
                    gates_ps[:, base + g * BL : base + (g + 1) * BL],
                    wih_sb[:, g * K : (g + 1) * K], xs,
                    start=(t % 4 == 0 and g == 0), stop=False,
                    skip_group_check=True,
                )
            if t > 0:
                hprev = ht[(t - 1) % 2]
                for g in range(4):
                    nc.tensor.matmul(
                        gates_ps[:, base + g * BL : base + (g + 1) * BL],
                        whh_sb[:, g * K : (g + 1) * K], hprev[:],
                        start=False, stop=(g == 3), skip_group_check=True,
                    )
                # heads matmul for h_{t-1}: stationary = h (free 32 ->
                # out partitions 32), moving = [W_mu.T | W_sig.T].
                j = t - 1
                hbase = (j % HSLOTS) * 2 * OBS
                nc.tensor.matmul(
                    heads_ps[:, hbase : hbase + 2 * OBS],
                    hprev[:], whd_sb[:],
                    start=True, stop=True, skip_group_check=True,
                )
            sg = sgt[t % 2]
            nc.scalar.activation(
                sg[:], gates_ps[:, base : base + 4 * BL], AF.Sigmoid
            )
            cnew = ct[t % 2]
            # gate layout in sg: i | f | o | g'  with  g = 2*sigmoid-1
            nc.vector.tensor_scalar(
                g2[:], sg[:, 3 * BL : 4 * BL], 2.0, 1.0, OP.mult, OP.subtract
            )
            if t == 0:
                nc.vector.tensor_mul(cnew[:], sg[:, 0:BL], g2[:])
            else:
                cprev = ct[(t - 1) % 2]
                nc.vector.tensor_mul(ig[:], sg[:, 0:BL], g2[:])
                nc.vector.tensor_mul(fc[:], sg[:, BL : 2 * BL], cprev[:])
                nc.vector.tensor_add(cnew[:], ig[:], fc[:])
            th = tht[t % 2]
            nc.scalar.activation(th[:], cnew[:], AF.Tanh)
            nc.vector.tensor_mul(ht[t % 2][:], sg[:, 2 * BL : 3 * BL], th[:])
            # flush completed 4-step heads groups: PSUM -> SBUF staging on
            # the otherwise-idle GPSIMD engine, then DMA to DRAM.
            j = t - 1
            if t > 0 and j % 4 == 3:
                s0 = ((j - 3) % HSLOTS) * 2 * OBS
                stg = stgt[(j // 4) % 2]
                if (j // 4) % 2 == 0:
                    nc.vector.tensor_copy(
                        stg[:], heads_ps[:, s0 : s0 + 4 * 2 * OBS]
                    )
                else:
                    nc.scalar.copy(
                        stg[:], heads_ps[:, s0 : s0 + 4 * 2 * OBS]
                    )
                nc.sync.dma_start(
                    heads[:, (j - 3) * 2 * OBS : (j + 1) * 2 * OBS], stg[:]
                )
                last_flush = j

        # final heads matmul + tail flush
        j = nsteps - 1
        hbase = (j % HSLOTS) * 2 * OBS
        nc.tensor.matmul(
            heads_ps[:, hbase : hbase + 2 * OBS],
            ht[j % 2][:], whd_sb[:],
            start=True, stop=True, skip_group_check=True,
        )
        f0 = last_flush + 1
        s0 = (f0 % HSLOTS) * 2 * OBS
        n = nsteps - f0
        stg = stgt[(f0 // 4) % 2]
        nc.vector.tensor_copy(
            stg[:, : n * 2 * OBS], heads_ps[:, s0 : s0 + n * 2 * OBS]
        )
        nc.sync.dma_start(
            heads[:, f0 * 2 * OBS : nsteps * 2 * OBS], stg[:, : n * 2 * OBS]
        )
    nc.compile()
    return nc


def _prep_weights(W_ih, W_hh, b_ih, b_hh, W_mu, W_sig):
    # torch gate order in rows: i(0:K) f(K:2K) g(2K:3K) o(3K:4K)
    # reorder rows to (i, f, o, g); scale the g block by 2 so one sigmoid
    # covers all gates (tanh(a) = 2*sigmoid(2a) - 1).
    perm = np.r_[0:K, K : 2 * K, 3 * K : 4 * K, 2 * K : 3 * K]
    whh_t = np.ascontiguousarray(W_hh[perm].T, np.float32)          # [K, 4K]
    bias = (b_ih + b_hh)[perm].astype(np.float32)
    wih_t = np.concatenate(
        [W_ih[perm].T, bias[None, :]], axis=0
    ).astype(np.float32)                                            # [IN+1, 4K]
    whh_t[:, 3 * K :] *= 2.0
    wih_t[:, 3 * K :] *= 2.0
    wheads = np.concatenate([W_mu.T, W_sig.T], axis=1).astype(np.float32)
    return (
        whh_t.astype(np.float16),
        wih_t.astype(np.float16),
        wheads.astype(np.float16),
    )


def kernel(external_input_seq, W_ih, W_hh, b_ih, b_hh, W_mu, b_mu, W_sig, b_sig):
    nsteps = _LSTEPS
    x = np.asarray(external_input_seq, np.float32)[:nsteps]
    W_ih = np.asarray(W_ih, np.float32)
    W_hh = np.asarray(W_hh, np.float32)
    b_ih = np.asarray(b_ih, np.float32)
    b_hh = np.asarray(b_hh, np.float32)
    W_mu = np.asarray(W_mu, np.float32)
    b_mu = np.asarray(b_mu, np.float32)
    W_sig = np.asarray(W_sig, np.float32)
    b_sig = np.asarray(b_sig, np.float32)

    whh_t, wih_t, wheads = _prep_weights(W_ih, W_hh, b_ih, b_hh, W_mu, W_sig)

    if nsteps not in _cache:
        _cache[nsteps] = build_nc(nsteps)
    nc = _cache[nsteps]

    in_maps = []
    for c in range(NCORES):
        xc = x[:, c * BL : (c + 1) * BL, :]              # [nsteps, BL, IN]
        xt = np.empty((IN + 1, nsteps * BL), np.float16)
        xt[:IN] = xc.transpose(2, 0, 1).reshape(IN, nsteps * BL)
        xt[IN] = 1.0
        in_maps.append(
            {"xt": xt, "whh_t": whh_t, "wih_t": wih_t, "wheads": wheads}
        )

    res = run_bass_kernel_spmd(
        nc, in_maps, core_ids=list(range(NCORES)), **RUN_KW
    )
    global LAST_RESULT
    LAST_RESULT = res

    mu = np.empty((nsteps, B, OBS), np.float32)
    sig = np.empty((nsteps, B, OBS), np.float32)
    for c in range(NCORES):
        h = res.results[c]["heads"].reshape(BL, nsteps, 2 * OBS)
        mu[:, c * BL : (c + 1) * BL, :] = h[:, :, :OBS].transpose(1, 0, 2)
        sig[:, c * BL : (c + 1) * BL, :] = h[:, :, OBS:].transpose(1, 0, 2)
    mu += b_mu
    sig += b_sig
    return mu, sig


# revision 11
# speedup vs baseline: 1.0198x; 1.0198x over previous
"""DeepAR LSTM kernel for 8 Trainium2 NeuronCores.

Data-parallel over batch (256 -> 8 cores x 32). Latency-oriented design:
the recurrence is a serial chain PE -> ACT -> DVE -> ACT -> DVE per step,
so everything else is moved off that chain.

  * fp16 matmul operands everywhere (1 PE cycle/row vs 4 for fp32);
    PSUM accumulation stays fp32.
  * Gate pre-activations accumulate in a persistent 16-step PSUM ring
    [128, 16*4*32]. The x-side matmuls for step t depend only on x (DMA'd
    up front), so the PE executes them early; only the 4 h-side matmuls
    (start=False accumulate) sit on the critical path.
  * One sigmoid covers all 4 gates: the g-gate rows of W/bias are
    pre-scaled by 2 on the host, and tanh(a) = 2*sigmoid(2a) - 1 is fixed
    up in DVE with a single tensor_scalar op.
  * Bias is folded into the x-side matmul via a constant-1 row of x.
  * mu/logsigma heads run as one PE matmul per step (stationary = h_t),
    accumulating into a PSUM ring that is DMA'd straight to DRAM every
    4 steps; head biases are added on the host.
"""

import os
import sys
from contextlib import ExitStack

import numpy as np

sys.path.insert(0, "/opt/trn_rl_repo")

import concourse.bass as bass
import concourse.tile as tile
from concourse import bacc, mybir
from concourse.bass_utils import run_bass_kernel_spmd

L, B, IN, K, OBS = 1024, 256, 64, 128, 32
NCORES = 8
BL = B // NCORES   # 32 batch rows per core
SLOTS = 16         # gate-psum ring depth (steps)
HSLOTS = 16        # heads-psum ring depth (steps)

_LSTEPS = int(os.environ.get("KERNEL_LSTEPS", L))  # smoke-test override

F32 = mybir.dt.float32
F16 = mybir.dt.float16
AF = mybir.ActivationFunctionType
OP = mybir.AluOpType

_cache = {}
RUN_KW = {}         # test harness may inject trace=True/tmpdir
LAST_RESULT = None  # BassKernelResults of the most recent run


def build_nc(nsteps: int) -> bass.Bass:
    nc = bacc.Bacc(
        "TRN2", target_bir_lowering=False, debug=False, num_devices=NCORES
    )
    xt = nc.dram_tensor("xt", [IN + 1, nsteps * BL], F16, kind="ExternalInput")
    whh = nc.dram_tensor("whh_t", [K, 4 * K], F16, kind="ExternalInput")
    wih = nc.dram_tensor("wih_t", [IN + 1, 4 * K], F16, kind="ExternalInput")
    whd = nc.dram_tensor("wheads", [K, 2 * OBS], F16, kind="ExternalInput")
    heads = nc.dram_tensor(
        "heads", [BL, nsteps * 2 * OBS], F32, kind="ExternalOutput"
    )

    with ExitStack() as ctx:
        tc = ctx.enter_context(tile.TileContext(nc))
        singles = ctx.enter_context(tc.tile_pool(name="singles", bufs=1))
        gpsp = ctx.enter_context(tc.tile_pool(name="gps", bufs=1, space="PSUM"))
        hpsp = ctx.enter_context(tc.tile_pool(name="hps", bufs=1, space="PSUM"))
        dpsp = ctx.enter_context(tc.tile_pool(name="dps", bufs=1, space="PSUM"))

        whh_sb = singles.tile([K, 4 * K], F16)
        nc.sync.dma_start(whh_sb[:], whh[:])
        wih_sb = singles.tile([IN + 1, 4 * K], F16)
        nc.sync.dma_start(wih_sb[:], wih[:])
        whd_sb = singles.tile([K, 2 * OBS], F16)
        nc.sync.dma_start(whd_sb[:], whd[:])
        xt_sb = singles.tile([IN + 1, nsteps * BL], F16)
        nc.sync.dma_start(xt_sb[:], xt[:])

        sgt = [singles.tile([K, 4 * BL], F16, name=f"sg{i}") for i in range(2)]
        ct = [singles.tile([K, BL], F16, name=f"c{i}") for i in range(2)]
        tht = [singles.tile([K, BL], F16, name=f"th{i}") for i in range(2)]
        ht = [singles.tile([K, BL], F16, name=f"h{i}") for i in range(2)]
        g2 = singles.tile([K, BL], F16)
        ig = singles.tile([K, BL], F16)
        fc = singles.tile([K, BL], F16)
        stgt = [
            singles.tile([BL, 8 * 2 * OBS], F32, name=f"stg{i}")
            for i in range(2)
        ]

        gates_ps = gpsp.tile([K, SLOTS * 4 * BL], F32)    # 4 PSUM banks
        heads_ps = hpsp.tile([BL, HSLOTS * 2 * OBS], F32)  # 1 PSUM bank

        # A matmul can carry only ONE sync wait; make PE observe each DMA
        # semaphore via a throwaway 1x1 matmul so real matmuls never need
        # a DMA wait on top of a data-dependency wait.
        dummy_ps = dpsp.tile([1, 1], F32)
        absorb_state = {"first": True}

        def pe_absorb(tile_ap):
            nc.tensor.matmul(
                dummy_ps[:], tile_ap[0:1, 0:1], tile_ap[0:1, 0:1],
                start=absorb_state["first"], stop=False,
                skip_group_check=True,
            )
            absorb_state["first"] = False

        pe_absorb(whh_sb)
        pe_absorb(wih_sb)
        pe_absorb(whd_sb)
        pe_absorb(xt_sb)

        last_flush = -1
        for t in range(nsteps):
            base = (t % SLOTS) * 4 * BL
            xs = xt_sb[:, t * BL : (t + 1) * BL]
            # x-side matmuls: no h dependency -> execute early, off the
            # critical path. First one carries the WAR wait vs the sigmoid
            # read of this slot 16 steps ago.
            #
            # start=True marks the whole 2KB PSUM bank (4 slots) as
            # pending-zero; writes to marked bytes overwrite and clear the
            # mark, writes to cleared bytes accumulate. So assert start
            # only on the first matmul touching each bank (t%4==0, g==0);
            # every other x-matmul overwrites via its still-pending mark
            # and every h-matmul accumulates.
            for g in range(4):
                nc.tensor.matmul(
                    gates_ps[:, base + g * BL : base + (g + 1) * BL],
                    wih_sb[:, g * K : (g + 1) * K], xs,
                    start=(t % 4 == 0 and g == 0), stop=False,
                    skip_group_check=True,
                )
            if t > 0:
                hprev = ht[(t - 1) % 2]
                for g in range(4):
                    nc.tensor.matmul(
                        gates_ps[:, base + g * BL : base + (g + 1) * BL],
                        whh_sb[:, g * K : (g + 1) * K], hprev[:],
                        start=False, stop=(g == 3), skip_group_check=True,
                    )
                # heads matmul for h_{t-1}: stationary = h (free 32 ->
                # out partitions 32), moving = [W_mu.T | W_sig.T].
                j = t - 1
                hbase = (j % HSLOTS) * 2 * OBS
                nc.tensor.matmul(
                    heads_ps[:, hbase : hbase + 2 * OBS],
                    hprev[:], whd_sb[:],
                    start=True, stop=True, skip_group_check=True,
                )
            sg = sgt[t % 2]
            nc.scalar.activation(
                sg[:], gates_ps[:, base : base + 4 * BL], AF.Sigmoid
            )
            cnew = ct[t % 2]
            # gate layout in sg: i | f | o | g'  with  g = 2*sigmoid-1
            nc.vector.tensor_scalar(
                g2[:], sg[:, 3 * BL : 4 * BL], 2.0, 1.0, OP.mult, OP.subtract
            )
            if t == 0:
                nc.vector.tensor_mul(cnew[:], sg[:, 0:BL], g2[:])
            else:
                cprev = ct[(t - 1) % 2]
                nc.vector.tensor_mul(ig[:], sg[:, 0:BL], g2[:])
                nc.vector.tensor_mul(fc[:], sg[:, BL : 2 * BL], cprev[:])
                nc.vector.tensor_add(cnew[:], ig[:], fc[:])
            th = tht[t % 2]
            nc.scalar.activation(th[:], cnew[:], AF.Tanh)
            nc.vector.tensor_mul(ht[t % 2][:], sg[:, 2 * BL : 3 * BL], th[:])
            # flush completed 4-step heads groups: PSUM -> SBUF staging on
            # the otherwise-idle GPSIMD engine, then DMA to DRAM.
            j = t - 1
            if t > 0 and j % 8 == 7:
                s0 = ((j - 7) % HSLOTS) * 2 * OBS
                stg = stgt[(j // 8) % 2]
                if (j // 8) % 2 == 0:
                    nc.vector.tensor_copy(
                        stg[:], heads_ps[:, s0 : s0 + 8 * 2 * OBS]
                    )
                else:
                    nc.scalar.copy(
                        stg[:], heads_ps[:, s0 : s0 + 8 * 2 * OBS]
                    )
                nc.sync.dma_start(
                    heads[:, (j - 7) * 2 * OBS : (j + 1) * 2 * OBS], stg[:]
                )
                last_flush = j

        # final heads matmul + tail flush
        j = nsteps - 1
        hbase = (j % HSLOTS) * 2 * OBS
        nc.tensor.matmul(
            heads_ps[:, hbase : hbase + 2 * OBS],
            ht[j % 2][:], whd_sb[:],
            start=True, stop=True, skip_group_check=True,
        )
        f0 = last_flush + 1
        s0 = (f0 % HSLOTS) * 2 * OBS
        n = nsteps - f0
        stg = stgt[(f0 // 8) % 2]
        nc.vector.tensor_copy(
            stg[:, : n * 2 * OBS], heads_ps[:, s0 : s0 + n * 2 * OBS]
        )
        nc.sync.dma_start(
            heads[:, f0 * 2 * OBS : nsteps * 2 * OBS], stg[:, : n * 2 * OBS]
        )
    nc.compile()
    return nc


def _prep_weights(W_ih, W_hh, b_ih, b_hh, W_mu, W_sig):
    # torch gate order in rows: i(0:K) f(K:2K) g(2K:3K) o(3K:4K)
    # reorder rows to (i, f, o, g); scale the g block by 2 so one sigmoid
    # covers all gates (tanh(a) = 2*sigmoid(2a) - 1).
    perm = np.r_[0:K, K : 2 * K, 3 * K : 4 * K, 2 * K : 3 * K]
    whh_t = np.ascontiguousarray(W_hh[perm].T, np.float32)          # [K, 4K]
    bias = (b_ih + b_hh)[perm].astype(np.float32)
    wih_t = np.concatenate(
        [W_ih[perm].T, bias[None, :]], axis=0
    ).astype(np.float32)                                            # [IN+1, 4K]
    whh_t[:, 3 * K :] *= 2.0
    wih_t[:, 3 * K :] *= 2.0
    wheads = np.concatenate([W_mu.T, W_sig.T], axis=1).astype(np.float32)
    return (
        whh_t.astype(np.float16),
        wih_t.astype(np.float16),
        wheads.astype(np.float16),
    )


def kernel(external_input_seq, W_ih, W_hh, b_ih, b_hh, W_mu, b_mu, W_sig, b_sig):
    nsteps = _LSTEPS
    x = np.asarray(external_input_seq, np.float32)[:nsteps]
    W_ih = np.asarray(W_ih, np.float32)
    W_hh = np.asarray(W_hh, np.float32)
    b_ih = np.asarray(b_ih, np.float32)
    b_hh = np.asarray(b_hh, np.float32)
    W_mu = np.asarray(W_mu, np.float32)
    b_mu = np.asarray(b_mu, np.float32)
    W_sig = np.asarray(W_sig, np.float32)
    b_sig = np.asarray(b_sig, np.float32)

    whh_t, wih_t, wheads = _prep_weights(W_ih, W_hh, b_ih, b_hh, W_mu, W_sig)

    if nsteps not in _cache:
        _cache[nsteps] = build_nc(nsteps)
    nc = _cache[nsteps]

    in_maps = []
    for c in range(NCORES):
        xc = x[:, c * BL : (c + 1) * BL, :]              # [nsteps, BL, IN]
        xt = np.empty((IN + 1, nsteps * BL), np.float16)
        xt[:IN] = xc.transpose(2, 0, 1).reshape(IN, nsteps * BL)
        xt[IN] = 1.0
        in_maps.append(
            {"xt": xt, "whh_t": whh_t, "wih_t": wih_t, "wheads": wheads}
        )

    res = run_bass_kernel_spmd(
        nc, in_maps, core_ids=list(range(NCORES)), **RUN_KW
    )
    global LAST_RESULT
    LAST_RESULT = res

    mu = np.empty((nsteps, B, OBS), np.float32)
    sig = np.empty((nsteps, B, OBS), np.float32)
    for c in range(NCORES):
        h = res.results[c]["heads"].reshape(BL, nsteps, 2 * OBS)
        mu[:, c * BL : (c + 1) * BL, :] = h[:, :, :OBS].transpose(1, 0, 2)
        sig[:, c * BL : (c + 1) * BL, :] = h[:, :, OBS:].transpose(1, 0, 2)
    mu += b_mu
    sig += b_sig
    return mu, sig
